# revision 10
# baseline (speedup 1.0000x reference)
# DiGCN Inception Block (2 blocks, 3 branches each) on 8 TRN2 NeuronCores.
#
# Math: each DIGCNConv branch is  segsum_dst(ew * (x @ W)[src]) + b.
# Matmul commutes with the weighted segment-sum, so per block we aggregate the
# RAW node features once per edge weight vector:
#   A1 = segsum_dst(ew1 * x[src]),  A2 = segsum_dst(ew2 * x[src])
#   block_out = x @ ln_w + A1 @ c1_w + A2 @ c2_w + (ln_b + c1_b + c2_b)
#
# Sharding: nodes and their incoming edges (partitioned by dst) across 8 cores.
# Per core, edges are sorted by (node-tile of dst, src-chunk) and padded so the
# SPMD program is uniform across cores. The gather of x[src] uses dma_gather
# (int16 indices, so the source table is viewed in 4 chunks of 25000 rows with
# per-instruction base offsets). The segment-sum runs on the TensorEngine:
# S^T[e, n] = (dst_rel[e] == n) built by a DVE iota-compare, then
# A^T[d, n] += msg[e, d]^T @ S^T[e, n] accumulated in PSUM per 128-node tile.
#
# Two launches: block1 produces x (gathered on host), block2 consumes it.

import os
import sys

for _p in ("/opt/trn_rl_repo", "/root/.axon_site/_ro/trn_rl_repo"):
    if os.path.isdir(_p) and _p not in sys.path:
        sys.path.insert(0, _p)
        break

import numpy as np
import ml_dtypes

import concourse.bacc as bacc
import concourse.tile as tile
import concourse.mybir as mybir
from concourse import bass_utils


class Cfg:
    def __init__(self, n, e, f_in, emb, out, chrows):
        self.N, self.E, self.F, self.EMB, self.OUT = n, e, f_in, emb, out
        self.M = 8                      # cores
        self.NPC = n // self.M          # nodes per core
        self.TILE = 128                 # nodes per node tile
        self.NTILES = -(-self.NPC // self.TILE)
        self.QUADS = -(-self.NTILES // 4)
        self.NTILES_PAD = self.QUADS * 4
        self.NPC_PAD = self.NTILES * self.TILE
        self.CHROWS = chrows            # gather-table chunk rows (int16 range)
        self.NCHUNK = -(-n // chrows)


FULL = Cfg(100000, 1600000, 128, 64, 32, 25000)


_SKIP_PADS = False


def _prep_edges(cfg, src, dst, ew1, ew2, skip_pads=None):
    if skip_pads is None:
        skip_pads = _SKIP_PADS
    """Sort/pad edges into the uniform per-core layout. Returns aux array
    [M, QUADS, 128, AUXW] (f32; trailing cols are bitcast int16 gather
    indices) plus T_c (subtiles per (node-tile, chunk) group)."""
    M, NT, NC, TILE = cfg.M, cfg.NTILES_PAD, cfg.NCHUNK, cfg.TILE
    src = src.astype(np.int64).ravel()
    dst = dst.astype(np.int64).ravel()
    core = dst // cfg.NPC
    rel = dst - core * cfg.NPC
    tl = rel // TILE
    ch = src // cfg.CHROWS
    gid = ((core * NT) + tl) * NC + ch
    ngroups = M * NT * NC
    counts = np.bincount(gid, minlength=ngroups)
    T_c = max(1, int(-(-counts.max() // TILE)))
    cap = T_c * TILE
    starts = np.zeros(ngroups + 1, np.int64)
    np.cumsum(counts, out=starts[1:])
    order = np.argsort(gid, kind="stable")
    gs = gid[order]
    pos = np.arange(cfg.E, dtype=np.int64) - starts[gs]
    slot = gs * cap + pos

    tot = ngroups * cap
    slot_ch = (np.arange(tot, dtype=np.int64) // cap) % NC
    if skip_pads:
        # trailing -1 indices are skipped by dma_gather (no descriptors);
        # stale SBUF is masked by ew=0 (g tiles are memset on first use)
        p_src = slot_ch * cfg.CHROWS
        pos_in_grp = np.arange(tot, dtype=np.int64) % cap
        grp_cnt = np.repeat(counts.clip(min=1), cap)
        p_src = np.where(pos_in_grp < grp_cnt, p_src, -1 + slot_ch * cfg.CHROWS)
    else:
        p_src = slot_ch * cfg.CHROWS    # pads gather row 0 of their chunk
    p_ew1 = np.zeros(tot, np.float32)
    p_ew2 = np.zeros(tot, np.float32)
    p_dst = np.full(tot, -1.0, np.float32)
    p_src[slot] = src[order]
    p_ew1[slot] = ew1.ravel()[order]
    p_ew2[slot] = ew2.ravel()[order]
    p_dst[slot] = (rel[order] - tl[order] * TILE).astype(np.float32)

    NT_E = NC * T_c
    Q = cfg.QUADS

    def col_block(a):
        # [M, NT, NC, T_c, 128] -> [M, Q, 128, 4 * NT_E], col = i*NT_E + c*T_c + s
        v = a.reshape(M, NT, NC, T_c, TILE)
        v = v.transpose(0, 1, 4, 2, 3).reshape(M, NT, TILE, NT_E)
        v = v.reshape(M, Q, 4, TILE, NT_E).transpose(0, 1, 3, 2, 4)
        return np.ascontiguousarray(v.reshape(M, Q, TILE, 4 * NT_E), np.float32)

    A_ew1 = col_block(p_ew1)
    A_ew2 = col_block(p_ew2)
    A_dst = col_block(p_dst)

    # gather indices: per (node tile, chunk) a stream of J = T_c*128 local
    # idxs, int16-packed [128, J//16]: element [16r + j%16, j//16] = stream[j]
    loc = p_src - slot_ch * cfg.CHROWS
    assert loc.max() < 32768 and loc.min() >= -1
    J = cap
    st = loc.reshape(M, NT, NC, cap).astype(np.int16)
    st = st.reshape(M, NT, NC, J // 16, 16).swapaxes(-1, -2)   # [.., 16, J//16]
    st = np.broadcast_to(st[:, :, :, None], (M, NT, NC, 8, 16, J // 16))
    st = st.reshape(M, NT, NC, TILE, J // 16)
    st = st.transpose(0, 1, 3, 2, 4).reshape(M, NT, TILE, NC * (J // 16))
    st = st.reshape(M, Q, 4, TILE, NC * (J // 16)).transpose(0, 1, 3, 2, 4)
    st = np.ascontiguousarray(st).reshape(M, Q, TILE, 4 * NC * (J // 16))
    idx_f32 = st.view(np.float32)       # [M, Q, 128, 4*NC*J//32]

    aux = np.concatenate([A_ew1, A_ew2, A_dst, idx_f32], axis=3)
    return np.ascontiguousarray(aux), T_c


def _own_tiles(cfg, x_core, d):
    # [M, NPC, d] -> transposed per-tile lhsT blocks [M, NTILES, d, 128]
    M = cfg.M
    pad = np.zeros((M, cfg.NPC_PAD, d), np.float32)
    pad[:, : cfg.NPC] = x_core
    v = pad.reshape(M, cfg.NTILES, cfg.TILE, d).transpose(0, 1, 3, 2)
    return np.ascontiguousarray(v).astype(np.float16)


def _build_block(cfg, d_in, d_out, T_c, repeat=1, variant="full",
                 mm_bf16=True, table_width=None):
    """One inception block as a Bass SPMD program.
    Inputs:  table [N, d_in] (gather source, replicated),
             own   [NTILES, d_in, 128] (transposed own-node features),
             aux   [QUADS, 128, AUXW], wts [d_in, 3*d_out] (c1|c2|ln),
             rows  [128, 2*TILE] (cols 0:128 = iota row per partition,
                                  cols 128:128+d_out = summed bias, replicated)
    Output:  out [NPC_PAD, d_out]"""
    NC, TILE, Q = cfg.NCHUNK, cfg.TILE, cfg.QUADS
    TW = table_width or d_in                 # gather row width (512B target)
    NT_E = NC * T_c
    J = T_c * TILE                       # idxs per gather instruction
    AUXW = 12 * NT_E + 4 * NC * (J // 32)
    f32 = mybir.dt.float32
    mmdt = mybir.dt.float16 if mm_bf16 else f32

    nc = bacc.Bacc("TRN2", target_bir_lowering=False, debug=False,
                   num_devices=cfg.M)
    table = nc.dram_tensor("table", [cfg.N, TW], mmdt, kind="ExternalInput")
    own = nc.dram_tensor("own", [cfg.NTILES, d_in, TILE], mmdt,
                         kind="ExternalInput")
    aux = nc.dram_tensor("aux", [Q, TILE, AUXW], f32, kind="ExternalInput")
    wts = nc.dram_tensor("wts", [d_in, 3 * d_out], mmdt,
                         kind="ExternalInput")
    rows = nc.dram_tensor("rows", [TILE, 2 * TILE], f32,
                          kind="ExternalInput")
    out = nc.dram_tensor("out", [cfg.NPC_PAD, d_out], f32,
                         kind="ExternalOutput")

    EW1, EW2, DSTR, IDX0 = 0, 4 * NT_E, 8 * NT_E, 12 * NT_E

    with tile.TileContext(nc) as tc:
        with (
            tc.tile_pool(name="const", bufs=1) as cpool,
            tc.tile_pool(name="sb", bufs=2) as pool,
            tc.tile_pool(name="m2", bufs=4) as m2pool,
            tc.tile_pool(name="ps", bufs=2, space="PSUM") as psum,
        ):
            wts_t = cpool.tile([d_in, 3 * d_out], mmdt, tag="wts")
            nc.sync.dma_start(out=wts_t[:], in_=wts[:, :])
            rows_t = cpool.tile([TILE, 2 * TILE], f32, tag="rows")
            nc.sync.dma_start(out=rows_t[:], in_=rows[:, :])

            for q in [qq for _ in range(repeat) for qq in range(Q)]:
                aux_t = pool.tile([TILE, AUXW], f32, tag="aux")
                nc.sync.dma_start(out=aux_t[:], in_=aux[q])
                idxv = aux_t[:, IDX0:AUXW].bitcast(mybir.dt.int16)
                for i in range(4):
                    t = q * 4 + i
                    if t >= cfg.NTILES:
                        continue
                    g_t = pool.tile([TILE, NT_E * TW], mmdt, tag="g")
                    g3 = g_t[:].rearrange("p (c m d) -> p c m d",
                                          c=NC, d=TW)
                    g4 = g_t[:].rearrange("p (c s d) -> p c s d",
                                          c=NC, d=TW)
                    if t < 2:
                        nc.vector.memset(g_t[:], 0.0)
                    elif variant == "nogather":
                        nc.vector.memset(g_t[:, 0: d_in], 0.0)
                    if variant != "nogather":
                        for c in range(NC):
                            blk = (i * NC + c) * (J // 16)
                            nc.gpsimd.dma_gather(
                                out_ap=g3[:, c],
                                in_ap=table[c * cfg.CHROWS:, :],
                                idxs_ap=idxv[:, blk: blk + (J // 16)],
                                num_idxs=J,
                                num_idxs_reg=J,
                                elem_size=TW,
                            )
                    if variant == "gatheronly":
                        xs = pool.tile([TILE, d_out], f32, tag="xs")
                        nc.vector.tensor_copy(xs[:], g_t[:, 0:d_out])
                        nc.sync.dma_start(
                            out=out[t * TILE:(t + 1) * TILE, :], in_=xs[:])
                        continue
                    dcol = aux_t[:, DSTR + i * NT_E: DSTR + (i + 1) * NT_E]
                    s_t = pool.tile([TILE, NT_E * TILE], mmdt, tag="S")
                    nc.vector.tensor_tensor(
                        out=s_t[:].rearrange("p (j n) -> p j n", n=TILE),
                        in0=dcol.unsqueeze(2).to_broadcast([TILE, NT_E, TILE]),
                        in1=rows_t[:, 0:TILE].unsqueeze(1).to_broadcast(
                            [TILE, NT_E, TILE]),
                        op=mybir.AluOpType.is_equal,
                    )
                    m1_t = pool.tile([TILE, NT_E * d_in], mmdt, tag="m1")
                    e1col = aux_t[:, EW1 + i * NT_E: EW1 + (i + 1) * NT_E]
                    nc.vector.tensor_tensor(
                        out=m1_t[:].rearrange("p (c s d) -> p c s d",
                                              c=NC, d=d_in),
                        in0=g4[:, :, :, 0:d_in],
                        in1=e1col.rearrange("p (c s) -> p c s", c=NC)
                            .unsqueeze(3).to_broadcast([TILE, NC, T_c, d_in]),
                        op=mybir.AluOpType.mult,
                    )
                    a1p = psum.tile([d_in, TILE], f32, tag="A1", space="PSUM")
                    a2p = psum.tile([d_in, TILE], f32, tag="A2", space="PSUM")
                    for jj in range(NT_E):
                        c, s = jj // T_c, jj % T_c
                        m2_t = m2pool.tile([TILE, d_in], mmdt, tag="m2")
                        nc.scalar.activation(
                            out=m2_t[:],
                            in_=g4[:, c, s, 0:d_in],
                            func=mybir.ActivationFunctionType.Copy,
                            scale=aux_t[:, EW2 + i * NT_E + jj:
                                        EW2 + i * NT_E + jj + 1],
                        )
                        nc.tensor.matmul(
                            out=a1p[:],
                            lhsT=m1_t[:, jj * d_in:(jj + 1) * d_in],
                            rhs=s_t[:, jj * TILE:(jj + 1) * TILE],
                            start=(jj == 0), stop=(jj == NT_E - 1),
                        )
                        nc.tensor.matmul(
                            out=a2p[:],
                            lhsT=m2_t[:],
                            rhs=s_t[:, jj * TILE:(jj + 1) * TILE],
                            start=(jj == 0), stop=(jj == NT_E - 1),
                        )
                    a1s = pool.tile([d_in, TILE], mmdt, tag="A1s")
                    a2s = pool.tile([d_in, TILE], mmdt, tag="A2s")
                    nc.vector.tensor_copy(a1s[:], a1p[:])
                    nc.vector.tensor_copy(a2s[:], a2p[:])
                    own_t = pool.tile([d_in, TILE], mmdt, tag="own")
                    nc.sync.dma_start(out=own_t[:], in_=own[t])
                    xp = psum.tile([TILE, d_out], f32, tag="x", space="PSUM")
                    nc.tensor.matmul(out=xp[:], lhsT=a1s[:],
                                     rhs=wts_t[:, 0:d_out],
                                     start=True, stop=False)
                    nc.tensor.matmul(out=xp[:], lhsT=a2s[:],
                                     rhs=wts_t[:, d_out:2 * d_out],
                                     start=False, stop=False)
                    nc.tensor.matmul(out=xp[:], lhsT=own_t[:],
                                     rhs=wts_t[:, 2 * d_out:3 * d_out],
                                     start=False, stop=True)
                    xs = pool.tile([TILE, d_out], f32, tag="xs")
                    nc.vector.tensor_tensor(
                        out=xs[:],
                        in0=xp[:],
                        in1=rows_t[:, TILE:TILE + d_out],
                        op=mybir.AluOpType.add,
                    )
                    nc.sync.dma_start(
                        out=out[t * TILE:(t + 1) * TILE, :], in_=xs[:])

    nc.compile()
    return nc


_BUILD_CACHE = {}


def _get_block(cfg, d_in, d_out, T_c):
    key = (cfg.N, cfg.E, d_in, d_out, T_c)
    if key not in _BUILD_CACHE:
        _BUILD_CACHE[key] = _build_block(
            cfg, d_in, d_out, T_c, table_width=max(d_in, 256 // 2))
    return _BUILD_CACHE[key]


def _run_block(cfg, ncb, table, own, aux, wts, rows):
    in_maps = []
    for c in range(cfg.M):
        in_maps.append({
            "table": table,
            "own": own[c],
            "aux": aux[c],
            "wts": wts,
            "rows": rows,
        })
    res = bass_utils.run_bass_kernel_spmd(
        ncb, in_maps, core_ids=list(range(cfg.M)))
    return np.stack([r["out"] for r in res.results])   # [M, NPC_PAD, d_out]


def _kernel_cfg(cfg, features, ew1, ew2, src, dst,
                ln1_w, ln1_b, c11_w, c11_b, c12_w, c12_b,
                ln2_w, ln2_b, c21_w, c21_b, c22_w, c22_b):
    features = np.ascontiguousarray(features, np.float32)
    aux, T_c = _prep_edges(cfg, src, dst, ew1, ew2)

    wts1 = np.ascontiguousarray(
        np.concatenate([c11_w, c12_w, ln1_w], axis=1),
        np.float32).astype(np.float16)
    rows1 = np.zeros((cfg.TILE, 2 * cfg.TILE), np.float32)
    rows1[:, : cfg.TILE] = np.arange(cfg.TILE)[None, :]
    rows1[:, cfg.TILE: cfg.TILE + cfg.EMB] = (
        np.asarray(ln1_b) + np.asarray(c11_b) + np.asarray(c12_b))[None, :]
    own1 = _own_tiles(cfg, features.reshape(cfg.M, cfg.NPC, cfg.F), cfg.F)

    feats16 = features.astype(np.float16)
    nc1 = _get_block(cfg, cfg.F, cfg.EMB, T_c)
    x_pad = _run_block(cfg, nc1, feats16, own1, aux, wts1, rows1)
    x_full = np.ascontiguousarray(
        x_pad[:, : cfg.NPC].reshape(cfg.N, cfg.EMB))

    wts2 = np.ascontiguousarray(
        np.concatenate([c21_w, c22_w, ln2_w], axis=1),
        np.float32).astype(np.float16)
    rows2 = np.zeros((cfg.TILE, 2 * cfg.TILE), np.float32)
    rows2[:, : cfg.TILE] = np.arange(cfg.TILE)[None, :]
    rows2[:, cfg.TILE: cfg.TILE + cfg.OUT] = (
        np.asarray(ln2_b) + np.asarray(c21_b) + np.asarray(c22_b))[None, :]
    own2 = _own_tiles(cfg, x_full.reshape(cfg.M, cfg.NPC, cfg.EMB), cfg.EMB)

    x_table = np.zeros((cfg.N, cfg.F), np.float16)
    x_table[:, : cfg.EMB] = x_full
    nc2 = _get_block(cfg, cfg.EMB, cfg.OUT, T_c)
    y_pad = _run_block(cfg, nc2, x_table, own2, aux, wts2, rows2)
    return np.ascontiguousarray(y_pad[:, : cfg.NPC].reshape(cfg.N, cfg.OUT))


def kernel(features, ew1, ew2, src, dst,
           ln1_w, ln1_b, c11_w, c11_b, c12_w, c12_b,
           ln2_w, ln2_b, c21_w, c21_b, c22_w, c22_b):
    return _kernel_cfg(FULL, features, ew1, ew2, src, dst,
                       ln1_w, ln1_b, c11_w, c11_b, c12_w, c12_b,
                       ln2_w, ln2_b, c21_w, c21_b, c22_w, c22_b)



# revision 11
# speedup vs baseline: 1.2360x; 1.2360x over previous
# DiGCN Inception Block (2 blocks, 3 branches each) on 8 TRN2 NeuronCores.
#
# Math: each DIGCNConv branch is  segsum_dst(ew * (x @ W)[src]) + b.
# Matmul commutes with the weighted segment-sum, so per block we aggregate the
# RAW node features once per edge weight vector:
#   A1 = segsum_dst(ew1 * x[src]),  A2 = segsum_dst(ew2 * x[src])
#   block_out = x @ ln_w + A1 @ c1_w + A2 @ c2_w + (ln_b + c1_b + c2_b)
#
# Sharding: nodes and their incoming edges (partitioned by dst) across 8 cores.
# Per core, edges are sorted by (node-tile of dst, src-chunk) and padded so the
# SPMD program is uniform across cores. The gather of x[src] uses dma_gather
# (int16 indices, so the source table is viewed in 4 chunks of 25000 rows with
# per-instruction base offsets). The segment-sum runs on the TensorEngine:
# S^T[e, n] = (dst_rel[e] == n) built by a DVE iota-compare, then
# A^T[d, n] += msg[e, d]^T @ S^T[e, n] accumulated in PSUM per 128-node tile.
#
# Two launches: block1 produces x (gathered on host), block2 consumes it.

import os
import sys

for _p in ("/opt/trn_rl_repo", "/root/.axon_site/_ro/trn_rl_repo"):
    if os.path.isdir(_p) and _p not in sys.path:
        sys.path.insert(0, _p)
        break

import numpy as np
import ml_dtypes

import concourse.bacc as bacc
import concourse.tile as tile
import concourse.mybir as mybir
from concourse import bass_utils


class Cfg:
    def __init__(self, n, e, f_in, emb, out, chrows):
        self.N, self.E, self.F, self.EMB, self.OUT = n, e, f_in, emb, out
        self.M = 8                      # cores
        self.NPC = n // self.M          # nodes per core
        self.TILE = 128                 # nodes per node tile
        self.NTILES = -(-self.NPC // self.TILE)
        self.QUADS = -(-self.NTILES // 4)
        self.NTILES_PAD = self.QUADS * 4
        self.NPC_PAD = self.NTILES * self.TILE
        self.CHROWS = chrows            # gather-table chunk rows (int16 range)
        self.NCHUNK = -(-n // chrows)


FULL = Cfg(100000, 1600000, 128, 64, 32, 25000)


_SKIP_PADS = False


def _prep_edges(cfg, src, dst, ew1, ew2, skip_pads=None):
    if skip_pads is None:
        skip_pads = _SKIP_PADS
    """Sort/pad edges into the uniform per-core layout. Returns aux array
    [M, QUADS, 128, AUXW] (f32; trailing cols are bitcast int16 gather
    indices) plus T_c (subtiles per (node-tile, chunk) group)."""
    M, NT, NC, TILE = cfg.M, cfg.NTILES_PAD, cfg.NCHUNK, cfg.TILE
    src = src.astype(np.int64).ravel()
    dst = dst.astype(np.int64).ravel()
    core = dst // cfg.NPC
    rel = dst - core * cfg.NPC
    tl = rel // TILE
    ch = src // cfg.CHROWS
    gid = ((core * NT) + tl) * NC + ch
    ngroups = M * NT * NC
    counts = np.bincount(gid, minlength=ngroups)
    T_c = max(1, int(-(-counts.max() // TILE)))
    cap = T_c * TILE
    starts = np.zeros(ngroups + 1, np.int64)
    np.cumsum(counts, out=starts[1:])
    order = np.argsort(gid, kind="stable")
    gs = gid[order]
    pos = np.arange(cfg.E, dtype=np.int64) - starts[gs]
    slot = gs * cap + pos

    tot = ngroups * cap
    slot_ch = (np.arange(tot, dtype=np.int64) // cap) % NC
    if skip_pads:
        # trailing -1 indices are skipped by dma_gather (no descriptors);
        # stale SBUF is masked by ew=0 (g tiles are memset on first use)
        p_src = slot_ch * cfg.CHROWS
        pos_in_grp = np.arange(tot, dtype=np.int64) % cap
        grp_cnt = np.repeat(counts.clip(min=1), cap)
        p_src = np.where(pos_in_grp < grp_cnt, p_src, -1 + slot_ch * cfg.CHROWS)
    else:
        p_src = slot_ch * cfg.CHROWS    # pads gather row 0 of their chunk
    p_ew1 = np.zeros(tot, np.float32)
    p_ew2 = np.zeros(tot, np.float32)
    p_dst = np.full(tot, -1.0, np.float32)
    p_src[slot] = src[order]
    p_ew1[slot] = ew1.ravel()[order]
    p_ew2[slot] = ew2.ravel()[order]
    p_dst[slot] = (rel[order] - tl[order] * TILE).astype(np.float32)

    NT_E = NC * T_c
    Q = cfg.QUADS

    def col_block(a):
        # [M, NT, NC, T_c, 128] -> [M, Q, 128, 4 * NT_E], col = i*NT_E + c*T_c + s
        v = a.reshape(M, NT, NC, T_c, TILE)
        v = v.transpose(0, 1, 4, 2, 3).reshape(M, NT, TILE, NT_E)
        v = v.reshape(M, Q, 4, TILE, NT_E).transpose(0, 1, 3, 2, 4)
        return np.ascontiguousarray(v.reshape(M, Q, TILE, 4 * NT_E), np.float32)

    A_ew1 = col_block(p_ew1)
    A_ew2 = col_block(p_ew2)
    A_dst = col_block(p_dst)

    # gather indices: per (node tile, chunk) a stream of J = T_c*128 local
    # idxs, int16-packed [128, J//16]: element [16r + j%16, j//16] = stream[j]
    loc = p_src - slot_ch * cfg.CHROWS
    assert loc.max() < 32768 and loc.min() >= -1
    J = cap
    st = loc.reshape(M, NT, NC, cap).astype(np.int16)
    st = st.reshape(M, NT, NC, J // 16, 16).swapaxes(-1, -2)   # [.., 16, J//16]
    st = np.broadcast_to(st[:, :, :, None], (M, NT, NC, 8, 16, J // 16))
    st = st.reshape(M, NT, NC, TILE, J // 16)
    st = st.transpose(0, 1, 3, 2, 4).reshape(M, NT, TILE, NC * (J // 16))
    st = st.reshape(M, Q, 4, TILE, NC * (J // 16)).transpose(0, 1, 3, 2, 4)
    st = np.ascontiguousarray(st).reshape(M, Q, TILE, 4 * NC * (J // 16))
    idx_f32 = st.view(np.float32)       # [M, Q, 128, 4*NC*J//32]

    aux = np.concatenate([A_ew1, A_ew2, A_dst, idx_f32], axis=3)
    return np.ascontiguousarray(aux), T_c


def _own_tiles(cfg, x_core, d):
    # [M, NPC, d] -> transposed per-tile lhsT blocks [M, NTILES, d, 128]
    M = cfg.M
    pad = np.zeros((M, cfg.NPC_PAD, d), np.float32)
    pad[:, : cfg.NPC] = x_core
    v = pad.reshape(M, cfg.NTILES, cfg.TILE, d).transpose(0, 1, 3, 2)
    return np.ascontiguousarray(v).astype(np.float16)


def _build_block(cfg, d_in, d_out, T_c, repeat=1, variant="full",
                 mm_bf16=True, table_width=None):
    """One inception block as a Bass SPMD program.
    Inputs:  table [N, d_in] (gather source, replicated),
             own   [NTILES, d_in, 128] (transposed own-node features),
             aux   [QUADS, 128, AUXW], wts [d_in, 3*d_out] (c1|c2|ln),
             rows  [128, 2*TILE] (cols 0:128 = iota row per partition,
                                  cols 128:128+d_out = summed bias, replicated)
    Output:  out [NPC_PAD, d_out]"""
    NC, TILE, Q = cfg.NCHUNK, cfg.TILE, cfg.QUADS
    TW = table_width or d_in                 # gather row width (512B target)
    NT_E = NC * T_c
    J = T_c * TILE                       # idxs per gather instruction
    AUXW = 12 * NT_E + 4 * NC * (J // 32)
    f32 = mybir.dt.float32
    mmdt = mybir.dt.float16 if mm_bf16 else f32

    nc = bacc.Bacc("TRN2", target_bir_lowering=False, debug=False,
                   num_devices=cfg.M)
    table = nc.dram_tensor("table", [cfg.N, TW], f32, kind="ExternalInput")
    own = nc.dram_tensor("own", [cfg.NTILES, d_in, TILE], mmdt,
                         kind="ExternalInput")
    aux = nc.dram_tensor("aux", [Q, TILE, AUXW], f32, kind="ExternalInput")
    wts = nc.dram_tensor("wts", [d_in, 3 * d_out], mmdt,
                         kind="ExternalInput")
    rows = nc.dram_tensor("rows", [TILE, 2 * TILE], f32,
                          kind="ExternalInput")
    out = nc.dram_tensor("out", [cfg.NPC_PAD, d_out], f32,
                         kind="ExternalOutput")

    EW1, EW2, DSTR, IDX0 = 0, 4 * NT_E, 8 * NT_E, 12 * NT_E

    with tile.TileContext(nc) as tc:
        with (
            tc.tile_pool(name="const", bufs=1) as cpool,
            tc.tile_pool(name="sb", bufs=2) as pool,
            tc.tile_pool(name="m2", bufs=4) as m2pool,
            tc.tile_pool(name="ps", bufs=2, space="PSUM") as psum,
        ):
            wts_t = cpool.tile([d_in, 3 * d_out], mmdt, tag="wts")
            nc.sync.dma_start(out=wts_t[:], in_=wts[:, :])
            rows_t = cpool.tile([TILE, 2 * TILE], f32, tag="rows")
            nc.sync.dma_start(out=rows_t[:], in_=rows[:, :])

            for q in [qq for _ in range(repeat) for qq in range(Q)]:
                aux_t = pool.tile([TILE, AUXW], f32, tag="aux")
                nc.sync.dma_start(out=aux_t[:], in_=aux[q])
                idxv = aux_t[:, IDX0:AUXW].bitcast(mybir.dt.int16)
                for i in range(4):
                    t = q * 4 + i
                    if t >= cfg.NTILES:
                        continue
                    g_t = pool.tile([TILE, NT_E * TW], f32, tag="g")
                    g3 = g_t[:].rearrange("p (c m d) -> p c m d",
                                          c=NC, d=TW)
                    g4 = g_t[:].rearrange("p (c s d) -> p c s d",
                                          c=NC, d=TW)
                    if t < 2:
                        nc.vector.memset(g_t[:], 0.0)
                    elif variant == "nogather":
                        nc.vector.memset(g_t[:, 0: d_in], 0.0)
                    if variant != "nogather":
                        for c in range(NC):
                            blk = (i * NC + c) * (J // 16)
                            nc.gpsimd.dma_gather(
                                out_ap=g3[:, c],
                                in_ap=table[c * cfg.CHROWS:, :],
                                idxs_ap=idxv[:, blk: blk + (J // 16)],
                                num_idxs=J,
                                num_idxs_reg=J,
                                elem_size=TW,
                            )
                    if variant == "gatheronly":
                        xs = pool.tile([TILE, d_out], f32, tag="xs")
                        nc.vector.tensor_copy(xs[:], g_t[:, 0:d_out])
                        nc.sync.dma_start(
                            out=out[t * TILE:(t + 1) * TILE, :], in_=xs[:])
                        continue
                    dcol = aux_t[:, DSTR + i * NT_E: DSTR + (i + 1) * NT_E]
                    s_t = pool.tile([TILE, NT_E * TILE], mmdt, tag="S")
                    nc.vector.tensor_tensor(
                        out=s_t[:].rearrange("p (j n) -> p j n", n=TILE),
                        in0=dcol.unsqueeze(2).to_broadcast([TILE, NT_E, TILE]),
                        in1=rows_t[:, 0:TILE].unsqueeze(1).to_broadcast(
                            [TILE, NT_E, TILE]),
                        op=mybir.AluOpType.is_equal,
                    )
                    m1_t = pool.tile([TILE, NT_E * d_in], mmdt, tag="m1")
                    e1col = aux_t[:, EW1 + i * NT_E: EW1 + (i + 1) * NT_E]
                    nc.vector.tensor_tensor(
                        out=m1_t[:].rearrange("p (c s d) -> p c s d",
                                              c=NC, d=d_in),
                        in0=g4[:, :, :, 0:d_in],
                        in1=e1col.rearrange("p (c s) -> p c s", c=NC)
                            .unsqueeze(3).to_broadcast([TILE, NC, T_c, d_in]),
                        op=mybir.AluOpType.mult,
                    )
                    a1p = psum.tile([d_in, TILE], f32, tag="A1", space="PSUM")
                    a2p = psum.tile([d_in, TILE], f32, tag="A2", space="PSUM")
                    for jj in range(NT_E):
                        c, s = jj // T_c, jj % T_c
                        m2_t = m2pool.tile([TILE, d_in], mmdt, tag="m2")
                        nc.scalar.activation(
                            out=m2_t[:],
                            in_=g4[:, c, s, 0:d_in],
                            func=mybir.ActivationFunctionType.Copy,
                            scale=aux_t[:, EW2 + i * NT_E + jj:
                                        EW2 + i * NT_E + jj + 1],
                        )
                        nc.tensor.matmul(
                            out=a1p[:],
                            lhsT=m1_t[:, jj * d_in:(jj + 1) * d_in],
                            rhs=s_t[:, jj * TILE:(jj + 1) * TILE],
                            start=(jj == 0), stop=(jj == NT_E - 1),
                        )
                        nc.tensor.matmul(
                            out=a2p[:],
                            lhsT=m2_t[:],
                            rhs=s_t[:, jj * TILE:(jj + 1) * TILE],
                            start=(jj == 0), stop=(jj == NT_E - 1),
                        )
                    a1s = pool.tile([d_in, TILE], mmdt, tag="A1s")
                    a2s = pool.tile([d_in, TILE], mmdt, tag="A2s")
                    nc.vector.tensor_copy(a1s[:], a1p[:])
                    nc.vector.tensor_copy(a2s[:], a2p[:])
                    own_t = pool.tile([d_in, TILE], mmdt, tag="own")
                    nc.sync.dma_start(out=own_t[:], in_=own[t])
                    xp = psum.tile([TILE, d_out], f32, tag="x", space="PSUM")
                    nc.tensor.matmul(out=xp[:], lhsT=a1s[:],
                                     rhs=wts_t[:, 0:d_out],
                                     start=True, stop=False)
                    nc.tensor.matmul(out=xp[:], lhsT=a2s[:],
                                     rhs=wts_t[:, d_out:2 * d_out],
                                     start=False, stop=False)
                    nc.tensor.matmul(out=xp[:], lhsT=own_t[:],
                                     rhs=wts_t[:, 2 * d_out:3 * d_out],
                                     start=False, stop=True)
                    xs = pool.tile([TILE, d_out], f32, tag="xs")
                    nc.vector.tensor_tensor(
                        out=xs[:],
                        in0=xp[:],
                        in1=rows_t[:, TILE:TILE + d_out],
                        op=mybir.AluOpType.add,
                    )
                    nc.sync.dma_start(
                        out=out[t * TILE:(t + 1) * TILE, :], in_=xs[:])

    nc.compile()
    return nc


_BUILD_CACHE = {}


def _get_block(cfg, d_in, d_out, T_c):
    key = (cfg.N, cfg.E, d_in, d_out, T_c)
    if key not in _BUILD_CACHE:
        _BUILD_CACHE[key] = _build_block(
            cfg, d_in, d_out, T_c, table_width=max(d_in, 512 // 4))
    return _BUILD_CACHE[key]


def _run_block(cfg, ncb, table, own, aux, wts, rows):
    in_maps = []
    for c in range(cfg.M):
        in_maps.append({
            "table": table,
            "own": own[c],
            "aux": aux[c],
            "wts": wts,
            "rows": rows,
        })
    res = bass_utils.run_bass_kernel_spmd(
        ncb, in_maps, core_ids=list(range(cfg.M)))
    return np.stack([r["out"] for r in res.results])   # [M, NPC_PAD, d_out]


def _kernel_cfg(cfg, features, ew1, ew2, src, dst,
                ln1_w, ln1_b, c11_w, c11_b, c12_w, c12_b,
                ln2_w, ln2_b, c21_w, c21_b, c22_w, c22_b):
    features = np.ascontiguousarray(features, np.float32)
    aux, T_c = _prep_edges(cfg, src, dst, ew1, ew2)

    wts1 = np.ascontiguousarray(
        np.concatenate([c11_w, c12_w, ln1_w], axis=1),
        np.float32).astype(np.float16)
    rows1 = np.zeros((cfg.TILE, 2 * cfg.TILE), np.float32)
    rows1[:, : cfg.TILE] = np.arange(cfg.TILE)[None, :]
    rows1[:, cfg.TILE: cfg.TILE + cfg.EMB] = (
        np.asarray(ln1_b) + np.asarray(c11_b) + np.asarray(c12_b))[None, :]
    own1 = _own_tiles(cfg, features.reshape(cfg.M, cfg.NPC, cfg.F), cfg.F)

    nc1 = _get_block(cfg, cfg.F, cfg.EMB, T_c)
    x_pad = _run_block(cfg, nc1, features, own1, aux, wts1, rows1)
    x_full = np.ascontiguousarray(
        x_pad[:, : cfg.NPC].reshape(cfg.N, cfg.EMB))

    wts2 = np.ascontiguousarray(
        np.concatenate([c21_w, c22_w, ln2_w], axis=1),
        np.float32).astype(np.float16)
    rows2 = np.zeros((cfg.TILE, 2 * cfg.TILE), np.float32)
    rows2[:, : cfg.TILE] = np.arange(cfg.TILE)[None, :]
    rows2[:, cfg.TILE: cfg.TILE + cfg.OUT] = (
        np.asarray(ln2_b) + np.asarray(c21_b) + np.asarray(c22_b))[None, :]
    own2 = _own_tiles(cfg, x_full.reshape(cfg.M, cfg.NPC, cfg.EMB), cfg.EMB)

    x_table = np.zeros((cfg.N, cfg.F), np.float32)
    x_table[:, : cfg.EMB] = x_full
    nc2 = _get_block(cfg, cfg.EMB, cfg.OUT, T_c)
    y_pad = _run_block(cfg, nc2, x_table, own2, aux, wts2, rows2)
    return np.ascontiguousarray(y_pad[:, : cfg.NPC].reshape(cfg.N, cfg.OUT))


def kernel(features, ew1, ew2, src, dst,
           ln1_w, ln1_b, c11_w, c11_b, c12_w, c12_b,
           ln2_w, ln2_b, c21_w, c21_b, c22_w, c22_b):
    return _kernel_cfg(FULL, features, ew1, ew2, src, dst,
                       ln1_w, ln1_b, c11_w, c11_b, c12_w, c12_b,
                       ln2_w, ln2_b, c21_w, c21_b, c22_w, c22_b)

